# revision 5
# baseline (speedup 1.0000x reference)
"""CrossEntropyLossByFrequencyTier on 8 trn2 NeuronCores (Bass/Tile).

Full inputs -> full outputs. Data-parallel over the token dim: each of the
8 cores gets 512 tokens x 50257 vocab (f32), computes per-token CE
(streamed logsumexp via ACT exp+accumulate, label logit via indirect DMA
gather), bins tokens into 4 frequency tiers with a one-hot mask matmul,
and emits a [4, 2] (value_sum, count) partial. Host sums partials across
cores and applies the empty-tier count=1 substitution.
"""

from contextlib import ExitStack

import numpy as np

import concourse.bass as bass
import concourse.tile as tile
from concourse import bacc, mybir
from concourse.bass_utils import run_bass_kernel_spmd
from concourse.hw_specs import get_activation_tables as _orig_act_tables

N = 4096
VOCAB = 50257
N_CORES = 8
TOK = N // N_CORES            # 512 tokens per core
P = 128                       # SBUF partitions
BLOCKS = TOK // P             # 4 token blocks per core
CHUNK = 8192                  # vocab chunk (free dim) per stream tile
N_FULL = VOCAB // CHUNK       # 6 full chunks
TAIL = VOCAB - N_FULL * CHUNK  # 1105
# Last block tapers off so the ACT engine's final-chunk lag after the
# stream ends is small.
CHUNKS_STD = [CHUNK] * N_FULL + [TAIL]
CHUNKS_LAST = [CHUNK] * (N_FULL - 1) + [4096, 2048, 1536, 1024, 593]
assert sum(CHUNKS_STD) == VOCAB and sum(CHUNKS_LAST) == VOCAB
TIER_BOUNDS = (100.0, 1000.0, 10000.0)
NT = len(TIER_BOUNDS) + 1     # 4 tiers

DEBUG_LOSSES = False          # also emit per-token losses (dev only)

_NC = None
LAST_RESULTS = None  # test harness introspection


def _patched_act_tables(arch):
    # Force Exp and Ln to resolve to the one table set containing both, so
    # the final Ln doesn't pay a ~2.5us ACT table swap after the stream.
    tables = {k: set(v) for k, v in _orig_act_tables(arch).items()}
    both = {mybir.ActivationFunctionType.Exp, mybir.ActivationFunctionType.Ln}
    if "natural_log_exp_and_others" in tables and \
            both <= tables["natural_log_exp_and_others"]:
        for name, funcs in tables.items():
            if name != "natural_log_exp_and_others":
                funcs -= both
    return tables


def _build():
    global _NC
    if _NC is not None:
        return _NC
    bacc.get_activation_tables = _patched_act_tables
    nc = bacc.Bacc("TRN2", target_bir_lowering=False, debug=False,
                   num_devices=N_CORES)
    f32 = mybir.dt.float32
    x = nc.dram_tensor("x", [TOK, VOCAB], f32, kind="ExternalInput")
    idx = nc.dram_tensor("idx", [TOK, 1], mybir.dt.int32, kind="ExternalInput")
    lab = nc.dram_tensor("lab", [TOK, 1], f32, kind="ExternalInput")
    partials = nc.dram_tensor("partials", [NT, 2], f32, kind="ExternalOutput")
    if DEBUG_LOSSES:
        losses = nc.dram_tensor("losses", [TOK, 1], f32,
                                kind="ExternalOutput")

    xa = x[:]
    xflat = xa.rearrange("a (b c) -> (a b) c", c=1)

    with tile.TileContext(nc) as tc, ExitStack() as ctx:
        xs = ctx.enter_context(tc.tile_pool(name="xs", bufs=5))
        accp = ctx.enter_context(tc.tile_pool(name="acc", bufs=BLOCKS))
        small = ctx.enter_context(tc.tile_pool(name="small", bufs=1))
        maskp = ctx.enter_context(tc.tile_pool(name="masks", bufs=2))
        psp = ctx.enter_context(tc.tile_pool(name="ps", bufs=1, space="PSUM"))

        s_all = small.tile([P, BLOCKS], f32, tag="s_all")
        logz = small.tile([P, BLOCKS], f32, tag="logz")
        picked = small.tile([P, BLOCKS], f32, tag="picked")
        idx_all = small.tile([P, BLOCKS], mybir.dt.int32, tag="idx_all")
        lab_all = small.tile([P, BLOCKS], f32, tag="lab_all")
        G = small.tile([P, BLOCKS * NT], f32, tag="G")
        R = small.tile([P, BLOCKS * 2], f32, tag="R")

        # Small per-block loads, the label-logit gather, and tier masks go
        # through GpSimd/SWDGE so they don't occupy the Sync queue ahead of
        # the stream issues; they finish during the stream ramp.
        for b in range(BLOCKS):
            rows = slice(b * P, (b + 1) * P)
            nc.gpsimd.dma_start(idx_all[:, b:b + 1], idx[rows, :])
            nc.gpsimd.dma_start(lab_all[:, b:b + 1], lab[rows, :])
            nc.gpsimd.indirect_dma_start(
                out=picked[:, b:b + 1],
                out_offset=None,
                in_=xflat,
                in_offset=bass.IndirectOffsetOnAxis(ap=idx_all[:, b:b + 1],
                                                    axis=0),
            )
            lc = lab_all[:, b:b + 1]
            t = maskp.tile([P, 3], f32, tag="t")
            for k, bound in enumerate(TIER_BOUNDS):
                nc.vector.tensor_scalar(t[:, k:k + 1], lc, bound, None,
                                        mybir.AluOpType.is_ge)
            g0 = b * NT
            nc.vector.tensor_scalar(G[:, g0:g0 + 1], lc, TIER_BOUNDS[0], None,
                                    mybir.AluOpType.is_lt)
            nc.vector.tensor_sub(G[:, g0 + 1:g0 + 2], t[:, 0:1], t[:, 1:2])
            nc.vector.tensor_sub(G[:, g0 + 2:g0 + 3], t[:, 1:2], t[:, 2:3])
            nc.vector.tensor_copy(G[:, g0 + 3:g0 + 4], t[:, 2:3])
            nc.vector.memset(R[:, 2 * b + 1:2 * b + 2], 1.0)

        # Main stream: exp each [128 tokens x chunk] tile in place; ACT
        # accumulates the per-token partial sum as a side output.
        for b in range(BLOCKS):
            rows = slice(b * P, (b + 1) * P)
            chunks = CHUNKS_LAST if b == BLOCKS - 1 else CHUNKS_STD
            acc = accp.tile([P, len(chunks)], f32, tag="acc")
            c0 = 0
            for c, w in enumerate(chunks):
                xt = xs.tile([P, w], f32, tag="xt")
                nc.sync.dma_start(xt[:, :w], xa[rows, c0:c0 + w])
                nc.scalar.activation(xt[:, :w], xt[:, :w],
                                     mybir.ActivationFunctionType.Exp,
                                     accum_out=acc[:, c:c + 1])
                c0 += w
            nc.vector.reduce_sum(s_all[:, b:b + 1], acc[:],
                                 axis=mybir.AxisListType.X)

        # log of the summed exps for all 4 blocks in one ACT call.
        nc.scalar.activation(logz[:], s_all[:],
                             mybir.ActivationFunctionType.Ln)

        ps = psp.tile([NT, 2], f32, tag="ps")
        for b in range(BLOCKS):
            rows = slice(b * P, (b + 1) * P)
            lcol = R[:, 2 * b:2 * b + 1]
            nc.vector.tensor_sub(lcol, logz[:, b:b + 1], picked[:, b:b + 1])
            if DEBUG_LOSSES:
                nc.sync.dma_start(losses[rows, :], lcol)
            # G_b.T @ [loss_b, 1] accumulated over blocks -> [4, 2]
            nc.tensor.matmul(out=ps[:], lhsT=G[:, b * NT:(b + 1) * NT],
                             rhs=R[:, 2 * b:2 * b + 2],
                             start=(b == 0), stop=(b == BLOCKS - 1))

        out_sb = small.tile([NT, 2], f32, tag="out_sb")
        nc.vector.tensor_copy(out_sb[:], ps[:])
        nc.sync.dma_start(partials[:], out_sb[:])

    nc.compile()
    _NC = nc
    return nc


def kernel(inputs: np.ndarray, labels: np.ndarray):
    global LAST_RESULTS
    nc = _build()
    inputs = np.ascontiguousarray(inputs, dtype=np.float32)
    lab64 = np.asarray(labels).astype(np.int64).reshape(N)

    in_maps = []
    local_rows = np.arange(TOK, dtype=np.int64) * VOCAB
    for c in range(N_CORES):
        sl = slice(c * TOK, (c + 1) * TOK)
        lab_c = lab64[sl]
        in_maps.append({
            "x": inputs[sl],
            "idx": (local_rows + lab_c).astype(np.int32).reshape(TOK, 1),
            "lab": lab_c.astype(np.float32).reshape(TOK, 1),
        })

    res = run_bass_kernel_spmd(nc, in_maps, core_ids=list(range(N_CORES)))
    LAST_RESULTS = res

    tot = np.zeros((NT, 2), dtype=np.float64)
    for r in res.results:
        tot += r["partials"].astype(np.float64)
    values = tot[:, 0].astype(np.float32)
    raw_counts = tot[:, 1]
    counts = np.where(raw_counts == 0, 1.0, raw_counts).astype(np.float32)
    return values, counts


# revision 19
# speedup vs baseline: 1.1309x; 1.1309x over previous
"""CrossEntropyLossByFrequencyTier on 8 trn2 NeuronCores (Bass/Tile).

Full inputs -> full outputs. Data-parallel over the token dim: each of the
8 cores gets 512 tokens x 50257 vocab (f32), computes per-token CE
(streamed logsumexp via ACT exp+accumulate, label logit via indirect DMA
gather), bins tokens into 4 frequency tiers with a one-hot mask matmul,
and emits a [4, 2] (value_sum, count) partial. Host sums partials across
cores and applies the empty-tier count=1 substitution.
"""

from contextlib import ExitStack

import numpy as np

import concourse.bass as bass
import concourse.tile as tile
from concourse import bacc, mybir
from concourse.bass_utils import run_bass_kernel_spmd
from concourse.hw_specs import get_activation_tables as _orig_act_tables

N = 4096
VOCAB = 50257
N_CORES = 8
TOK = N // N_CORES            # 512 tokens per core
P = 128                       # SBUF partitions
BLOCKS = TOK // P             # 4 token blocks per core
CHUNK = 8192                  # vocab chunk (free dim) per stream tile
N_FULL = VOCAB // CHUNK       # 6 full chunks
TAIL = VOCAB - N_FULL * CHUNK  # 1105
# Last block tapers off gradually (r~0.7) so the ACT engine's exp backlog
# when the stream ends is small: ACT lags each chunk by ~its own exp time,
# so the suffix sum of (exp_j - dma_j) stays small instead of a full
# 8192-chunk exp (~7us).
CHUNKS_STD = [CHUNK] * N_FULL + [TAIL]
CHUNKS_LAST = [CHUNK] * (N_FULL - 2) + [5565, 3896, 2727, 1909, 1336, 936,
                                        655, 465]
assert sum(CHUNKS_STD) == VOCAB and sum(CHUNKS_LAST) == VOCAB
TIER_BOUNDS = (100.0, 1000.0, 10000.0)
NT = len(TIER_BOUNDS) + 1     # 4 tiers

DEBUG_LOSSES = False          # also emit per-token losses (dev only)

_NC = None
LAST_RESULTS = None  # test harness introspection


def _patched_act_tables(arch):
    # Force Exp and Ln to resolve to the one table set containing both, so
    # the final Ln doesn't pay a ~2.5us ACT table swap after the stream.
    tables = {k: set(v) for k, v in _orig_act_tables(arch).items()}
    both = {mybir.ActivationFunctionType.Exp, mybir.ActivationFunctionType.Ln}
    if "natural_log_exp_and_others" in tables and \
            both <= tables["natural_log_exp_and_others"]:
        for name, funcs in tables.items():
            if name != "natural_log_exp_and_others":
                funcs -= both
    return tables


def _build():
    global _NC
    if _NC is not None:
        return _NC
    bacc.get_activation_tables = _patched_act_tables
    nc = bacc.Bacc("TRN2", target_bir_lowering=False, debug=False,
                   num_devices=N_CORES)
    f32 = mybir.dt.float32
    x = nc.dram_tensor("x", [TOK, VOCAB], f32, kind="ExternalInput")
    idx = nc.dram_tensor("idx", [TOK, 1], mybir.dt.int32, kind="ExternalInput")
    lab = nc.dram_tensor("lab", [TOK, 1], f32, kind="ExternalInput")
    partials = nc.dram_tensor("partials", [NT, 2], f32, kind="ExternalOutput")
    if DEBUG_LOSSES:
        losses = nc.dram_tensor("losses", [TOK, 1], f32,
                                kind="ExternalOutput")

    xa = x[:]
    xflat = xa.rearrange("a (b c) -> (a b) c", c=1)

    with tile.TileContext(nc) as tc, ExitStack() as ctx:
        xs = ctx.enter_context(tc.tile_pool(name="xs", bufs=5))
        accp = ctx.enter_context(tc.tile_pool(name="acc", bufs=BLOCKS))
        small = ctx.enter_context(tc.tile_pool(name="small", bufs=1))
        maskp = ctx.enter_context(tc.tile_pool(name="masks", bufs=2))
        psp = ctx.enter_context(tc.tile_pool(name="ps", bufs=1, space="PSUM"))

        s_all = small.tile([P, BLOCKS], f32, tag="s_all")
        logz = small.tile([P, BLOCKS], f32, tag="logz")
        picked = small.tile([P, BLOCKS], f32, tag="picked")
        idx_all = small.tile([P, BLOCKS], mybir.dt.int32, tag="idx_all")
        lab_all = small.tile([P, BLOCKS], f32, tag="lab_all")
        G = small.tile([P, BLOCKS * NT], f32, tag="G")
        R = small.tile([P, BLOCKS * 2], f32, tag="R")

        # Small per-block loads, the label-logit gather, and tier masks go
        # through GpSimd/SWDGE so they issue immediately without occupying
        # the Sync queue; they complete during the stream ramp, so the tail
        # chain (loss -> matmul -> partials) never waits on a gather.
        for b in range(BLOCKS):
            rows = slice(b * P, (b + 1) * P)
            nc.gpsimd.dma_start(idx_all[:, b:b + 1], idx[rows, :])
            nc.gpsimd.dma_start(lab_all[:, b:b + 1], lab[rows, :])
            nc.gpsimd.indirect_dma_start(
                out=picked[:, b:b + 1],
                out_offset=None,
                in_=xflat,
                in_offset=bass.IndirectOffsetOnAxis(ap=idx_all[:, b:b + 1],
                                                    axis=0),
            )
            lc = lab_all[:, b:b + 1]
            t = maskp.tile([P, 3], f32, tag="t")
            for k, bound in enumerate(TIER_BOUNDS):
                nc.vector.tensor_scalar(t[:, k:k + 1], lc, bound, None,
                                        mybir.AluOpType.is_ge)
            g0 = b * NT
            nc.vector.tensor_scalar(G[:, g0:g0 + 1], lc, TIER_BOUNDS[0], None,
                                    mybir.AluOpType.is_lt)
            nc.vector.tensor_sub(G[:, g0 + 1:g0 + 2], t[:, 0:1], t[:, 1:2])
            nc.vector.tensor_sub(G[:, g0 + 2:g0 + 3], t[:, 1:2], t[:, 2:3])
            nc.vector.tensor_copy(G[:, g0 + 3:g0 + 4], t[:, 2:3])
            nc.vector.memset(R[:, 2 * b + 1:2 * b + 2], 1.0)

        # Main stream: exp each [128 tokens x chunk] tile in place; ACT
        # accumulates the per-token partial sum as a side output.
        for b in range(BLOCKS):
            rows = slice(b * P, (b + 1) * P)
            chunks = CHUNKS_LAST if b == BLOCKS - 1 else CHUNKS_STD
            acc = accp.tile([P, len(chunks)], f32, tag="acc")
            c0 = 0
            for c, w in enumerate(chunks):
                xt = xs.tile([P, w], f32, tag="xt")
                nc.sync.dma_start(xt[:, :w], xa[rows, c0:c0 + w])
                nc.scalar.activation(xt[:, :w], xt[:, :w],
                                     mybir.ActivationFunctionType.Exp,
                                     accum_out=acc[:, c:c + 1])
                c0 += w
            nc.vector.reduce_sum(s_all[:, b:b + 1], acc[:],
                                 axis=mybir.AxisListType.X)

        # log of the summed exps for all 4 blocks in one ACT call.
        nc.scalar.activation(logz[:], s_all[:],
                             mybir.ActivationFunctionType.Ln)

        ps = psp.tile([NT, 2], f32, tag="ps")
        for b in range(BLOCKS):
            rows = slice(b * P, (b + 1) * P)
            lcol = R[:, 2 * b:2 * b + 1]
            nc.vector.tensor_sub(lcol, logz[:, b:b + 1], picked[:, b:b + 1])
            if DEBUG_LOSSES:
                nc.sync.dma_start(losses[rows, :], lcol)
            # G_b.T @ [loss_b, 1] accumulated over blocks -> [4, 2]
            nc.tensor.matmul(out=ps[:], lhsT=G[:, b * NT:(b + 1) * NT],
                             rhs=R[:, 2 * b:2 * b + 2],
                             start=(b == 0), stop=(b == BLOCKS - 1))

        out_sb = small.tile([NT, 2], f32, tag="out_sb")
        nc.vector.tensor_copy(out_sb[:], ps[:])
        nc.sync.dma_start(partials[:], out_sb[:])

    nc.compile()
    _NC = nc
    return nc


def kernel(inputs: np.ndarray, labels: np.ndarray):
    global LAST_RESULTS
    nc = _build()
    inputs = np.ascontiguousarray(inputs, dtype=np.float32)
    lab64 = np.asarray(labels).astype(np.int64).reshape(N)

    in_maps = []
    local_rows = np.arange(TOK, dtype=np.int64) * VOCAB
    for c in range(N_CORES):
        sl = slice(c * TOK, (c + 1) * TOK)
        lab_c = lab64[sl]
        in_maps.append({
            "x": inputs[sl],
            "idx": (local_rows + lab_c).astype(np.int32).reshape(TOK, 1),
            "lab": lab_c.astype(np.float32).reshape(TOK, 1),
        })

    res = run_bass_kernel_spmd(nc, in_maps, core_ids=list(range(N_CORES)))
    LAST_RESULTS = res

    tot = np.zeros((NT, 2), dtype=np.float64)
    for r in res.results:
        tot += r["partials"].astype(np.float64)
    values = tot[:, 0].astype(np.float32)
    raw_counts = tot[:, 1]
    counts = np.where(raw_counts == 0, 1.0, raw_counts).astype(np.float32)
    return values, counts
